# revision 7
# baseline (speedup 1.0000x reference)
"""3-layer GCN (GCNConvNet) on 8 Trainium2 NeuronCores.

Math refactor (as the original baseline): with isd = 1/sqrt(deg+1) and
self-loops folded in, each layer becomes

    g      = isd**2 * relu(Q_prev)          (node-major "source features")
    P[n]   = sum_{e: dst(e)=n} g[src(e)]    (+ g[n] self term)
    Q[n]   = Waug^T @ [P[n]; sigma[n]]      (Waug = [W^T; b])
    h'     = isd * relu(Q)                  (final layer: out = isd * Q)

Per-edge coefficients disappear into per-node scaling; the scatter
matrices are pure one-hot, so the segment sum is a chain of PE matmuls
(gathered-msgs chunk as stationary, one-hot S as the 128-col stream).

v7 data layout (vs the old baseline):
  * The gather table is PACKED [25088 tokens, 128] f16 — one 256-byte
    token = TWO consecutive node rows of the [50176, 64] f16 table.  A
    token index is src>>1 (< 25088, fits int16), so there is a single
    index space (no A/B half-slab split); buckets are (dst tile, src
    parity) and the parity picks the 64-column half of the gathered
    chunk used as the matmul lhsT.  Halved table also doubles the DRAM
    row-buffer hit rate of the random gathers (~15.8 vs ~11.3 B/ns per
    SDMA engine measured).
  * The AllGather moves [6272, 64] per core (half the old padded rows),
    and the next layer gathers straight from its output.
  * Own-slab g rows and sigma live in SBUF (no per-tile gself reloads,
    no per-group sigma DMAs — the old kernel issued ~37k small HWDGE
    descriptors per run that competed with gather descriptors).
"""

import numpy as np

NC_CORES = 8
TILE = 128
GRP_TILES = 4
D_F = 64
WIN = 1024                       # slots per gather window (8 chunks)
N = 50000
OWN = N // NC_CORES              # 6250
NTILES = (OWN + TILE - 1) // TILE  # 49
OWNP = NTILES * TILE             # 6272 padded rows per core
TOKC = OWNP // 2                 # 3136 tokens per core
TOKT = NC_CORES * TOKC           # 25088 tokens in the AG'd table
ZID = TOKC - 1                   # core-0 pad token (zeroed rows 6270/6271)


def _wrap16(v):
    """[S] int -> [128, S//16] int16, index i at [i%16, i//16], replicated x8."""
    S = v.shape[0]
    assert S % 16 == 0
    w = v.reshape(S // 16, 16).T.astype(np.int16)
    return np.ascontiguousarray(np.tile(w, (8, 1)))


def _prepare(x, edge_index, W0, b0, W1, b1, W2, b2):
    x = np.asarray(x, dtype=np.float32)
    ei = np.asarray(edge_index)
    Ws = [np.asarray(W, np.float32) for W in (W0, W1, W2)]
    bs = [np.asarray(b, np.float32) for b in (b0, b1, b2)]

    src = ei[0].astype(np.int64)
    dst = ei[1].astype(np.int64)

    deg = np.bincount(dst, minlength=N).astype(np.float32) + 1.0
    isd = (1.0 / np.sqrt(deg)).astype(np.float32)
    sigma_full = (
        np.bincount(dst, weights=isd[src].astype(np.float64), minlength=N)
        .astype(np.float32) + isd
    )
    g0 = (isd[:, None] * x).astype(np.float16)            # [N, 64]

    # layer-0 packed token table [TOKT, 128] (core-padded rows zero)
    g0tok = np.zeros((TOKT, 128), np.float16)
    for c in range(NC_CORES):
        blk = g0[c * OWN:(c + 1) * OWN]
        g0tok[c * TOKC: c * TOKC + OWN // 2] = blk.reshape(OWN // 2, 128)

    tok_of_src = (src // OWN) * TOKC + (src % OWN) // 2

    # ---- edge bucketing: (core, tile, parity) ------------------------------
    core = dst // OWN
    dstl = dst % OWN
    tl = dstl // TILE
    par = src & 1
    key = (core * NTILES + tl) * 2 + par
    order = np.argsort(key, kind="stable")
    s_tok = tok_of_src[order]
    s_dl = (dstl % TILE)[order]
    counts = np.bincount(key, minlength=NC_CORES * NTILES * 2).reshape(
        NC_CORES, NTILES, 2)
    starts = np.zeros(NC_CORES * NTILES * 2 + 1, np.int64)
    np.cumsum(counts.reshape(-1), out=starts[1:])

    CP = (-(-counts.max(axis=0) // TILE)).astype(np.int64)   # [NTILES, 2]
    nmm_t = CP.sum(axis=1)
    chunk_base = np.zeros(NTILES + 1, np.int64)
    np.cumsum(nmm_t, out=chunk_base[1:])
    nchunk = int(chunk_base[-1])
    slots = nchunk * TILE
    s_pad = -(-slots // WIN) * WIN
    nwin = s_pad // WIN
    chunk_par = np.zeros(nchunk, np.int64)
    for t in range(NTILES):
        chunk_par[chunk_base[t] + CP[t, 0]: chunk_base[t + 1]] = 1

    per_core = []
    for c in range(NC_CORES):
        gflat = np.full(s_pad, ZID, np.int64)
        dflat = np.full(nchunk * TILE, -1.0, np.float32)
        for t in range(NTILES):
            for p in range(2):
                k = (c * NTILES + t) * 2 + p
                lo, hi = starts[k], starts[k + 1]
                n = hi - lo
                s0 = (chunk_base[t] + (CP[t, 0] if p else 0)) * TILE
                gflat[s0: s0 + n] = s_tok[lo:hi]
                dflat[s0: s0 + n] = s_dl[lo:hi]
        own_isd = isd[c * OWN:(c + 1) * OWN]
        isd2 = np.zeros(NTILES * TILE, np.float32)
        isd2[:OWN] = own_isd ** 2
        sig = np.zeros(OWNP, np.float16)
        sig[:OWN] = sigma_full[c * OWN:(c + 1) * OWN].astype(np.float16)
        gown0 = np.zeros((TILE, NTILES * D_F), np.float16)
        blk = g0[c * OWN:(c + 1) * OWN]                   # [6250, 64]
        for t in range(NTILES):
            r = min(TILE, OWN - t * TILE)
            gown0[:r, t * D_F:(t + 1) * D_F] = blk[t * TILE: t * TILE + r]
        per_core.append(dict(
            idx=_wrap16(gflat),
            dstl=np.ascontiguousarray(
                dflat.reshape(nchunk, TILE).T.astype(np.float16)),
            sigma=sig.reshape(1, OWNP),
            isd2=np.ascontiguousarray(isd2.reshape(NTILES, TILE).T),
            isdrow=isd[c * OWN:(c + 1) * OWN].astype(np.float32)
            .reshape(1, OWN),
            gown0=gown0,
        ))

    waug = []
    for W, b in zip(Ws, bs):
        wa = np.zeros((D_F + 1, W.shape[0]), np.float16)
        wa[:D_F, :] = W.T.astype(np.float16)
        wa[D_F, :] = b.astype(np.float16)
        waug.append(wa)

    iota = np.tile(np.arange(TILE, dtype=np.float16), (TILE, 1))
    ident = np.eye(TILE, dtype=np.float16)

    meta = dict(CP=CP, chunk_base=chunk_base, nchunk=nchunk, s_pad=s_pad,
                nwin=nwin, chunk_par=chunk_par, d_out=Ws[2].shape[0])

    in_maps = []
    for c in range(NC_CORES):
        m = dict(per_core[c])
        m["g0tok"] = g0tok
        for l in range(3):
            m[f"waug{l}"] = waug[l]
        m["iota"] = iota
        m["ident"] = ident
        in_maps.append(m)
    return meta, in_maps


# ----------------------------------------------------------------------------
# device kernel
# ----------------------------------------------------------------------------


def _build(meta, n_dev=NC_CORES):
    import concourse.bacc as bacc
    import concourse.mybir as mybir
    from concourse.tile import TileContext

    f16 = mybir.dt.float16
    f32 = mybir.dt.float32
    i16 = mybir.dt.int16

    CP = meta["CP"]
    chunk_base = meta["chunk_base"]
    nchunk = meta["nchunk"]
    s_pad = meta["s_pad"]
    nwin = meta["nwin"]
    chunk_par = meta["chunk_par"]
    d_out = meta["d_out"]

    ngrp = (NTILES + GRP_TILES - 1) // GRP_TILES
    grp_tiles = [list(range(g * GRP_TILES, min((g + 1) * GRP_TILES, NTILES)))
                 for g in range(ngrp)]
    max_ch = max(int(chunk_base[ts[-1] + 1] - chunk_base[ts[0]])
                 for ts in grp_tiles)

    nc = bacc.Bacc("TRN2", target_bir_lowering=False, num_devices=n_dev,
                   num_swdge_queues=4)

    g0tok_d = nc.dram_tensor("g0tok", [TOKT, 128], f16, kind="ExternalInput")
    gown0_d = nc.dram_tensor("gown0", [TILE, NTILES * D_F], f16,
                             kind="ExternalInput")
    idx_d = nc.dram_tensor("idx", [128, s_pad // 16], i16, kind="ExternalInput")
    dstl_d = nc.dram_tensor("dstl", [128, nchunk], f16, kind="ExternalInput")
    waug_d = [nc.dram_tensor(f"waug{l}", [D_F + 1, do], f16,
                             kind="ExternalInput")
              for l, do in enumerate([D_F, D_F, d_out])]
    sigma_d = nc.dram_tensor("sigma", [1, OWNP], f16, kind="ExternalInput")
    isd2_d = nc.dram_tensor("isd2", [TILE, NTILES], f32, kind="ExternalInput")
    isdrow_d = nc.dram_tensor("isdrow", [1, OWN], f32, kind="ExternalInput")
    iota_d = nc.dram_tensor("iota", [TILE, TILE], f16, kind="ExternalInput")
    ident_d = nc.dram_tensor("ident", [TILE, TILE], f16, kind="ExternalInput")
    out_d = nc.dram_tensor("out", [1, OWN], f32, kind="ExternalOutput")

    gown_d = [nc.dram_tensor(f"gown{l}", [OWNP, D_F], f16) for l in (1, 2)]
    gfull_d = [nc.dram_tensor(f"gfull{l}", [TOKT, 128], f16,
                              addr_space="Shared") for l in (1, 2)]
    rg = [list(range(NC_CORES))]

    with TileContext(nc) as tc:
        with (
            tc.tile_pool(name="static", bufs=1) as stp,
            tc.tile_pool(name="msgs", bufs=4) as mp,
            tc.tile_pool(name="smat", bufs=2) as sp,
            tc.tile_pool(name="paug", bufs=2) as pp,
            tc.tile_pool(name="qrelu", bufs=2) as qp,
            tc.tile_pool(name="pps", bufs=2, space="PSUM") as p_ps,
            tc.tile_pool(name="qps", bufs=2, space="PSUM") as q_ps,
            tc.tile_pool(name="tps", bufs=2, space="PSUM") as t_ps,
        ):
            reg_cache = {}
            qn = [0]

            def nreg(v):
                if v not in reg_cache:
                    r = nc.gpsimd.alloc_register(f"nidx{v}")
                    nc.gpsimd.reg_mov(r, v)
                    reg_cache[v] = r
                return reg_cache[v]

            iota_sb = stp.tile([TILE, TILE], f16)
            nc.sync.dma_start(out=iota_sb[:], in_=iota_d[:])
            ident_sb = stp.tile([TILE, TILE], f16)
            nc.sync.dma_start(out=ident_sb[:], in_=ident_d[:])
            ident32_sb = stp.tile([TILE, TILE], f32)
            nc.vector.tensor_copy(ident32_sb[:], ident_sb[:])
            waug_sb = []
            for l, do in enumerate([D_F, D_F, d_out]):
                w = stp.tile([D_F + 1, do], f16, tag=f"waug{l}")
                nc.sync.dma_start(out=w[:], in_=waug_d[l][:])
                waug_sb.append(w)
            isd2_sb = stp.tile([TILE, NTILES], f32)
            nc.sync.dma_start(out=isd2_sb[:], in_=isd2_d[:])
            isdrow_sb = stp.tile([1, OWN], f32)
            nc.sync.dma_start(out=isdrow_sb[:], in_=isdrow_d[:])
            sigma_sb = stp.tile([1, OWNP], f16)
            nc.sync.dma_start(out=sigma_sb[:], in_=sigma_d[:])
            idx_sb = stp.tile([128, s_pad // 16], i16)
            nc.sync.dma_start(out=idx_sb[:], in_=idx_d[:])
            dstl_sb = stp.tile([128, nchunk], f16)
            nc.sync.dma_start(out=dstl_sb[:], in_=dstl_d[:])
            out_sb = stp.tile([1, OWN], f32)
            zero_sb = stp.tile([OWNP - OWN, D_F], f16, tag="zeros")
            nc.vector.memset(zero_sb[:], 0.0)
            # resident own-slab g (node-major, [128, t*64+f]), two buffers
            gselfA = stp.tile([TILE, NTILES * D_F], f16, tag="gselfA")
            nc.sync.dma_start(out=gselfA[:], in_=gown0_d[:])
            gselfB = stp.tile([TILE, NTILES * D_F], f16, tag="gselfB")

            for layer in range(3):
                do = D_F if layer < 2 else d_out
                gsrc = [g0tok_d, gfull_d[0], gfull_d[1]][layer]
                gcur = gselfA if layer % 2 == 0 else gselfB
                gnxt = gselfB if layer % 2 == 0 else gselfA

                # ---- gathers of msg rows, streamed in max-size windows -----
                wins = []
                for w in range(nwin):
                    wt = mp.tile([128, WIN], f16, tag="win")
                    nc.gpsimd.dma_gather(
                        wt[:].rearrange("p (c e) -> p c e", e=TILE),
                        gsrc[:],
                        idx_sb[:, w * (WIN // 16):(w + 1) * (WIN // 16)],
                        WIN, nreg(WIN), TILE,
                        queue_num=qn[0],
                    )
                    qn[0] = (qn[0] + 1) % 4
                    wins.append(wt)

                def msg_lhs(ch):
                    wt = wins[ch // 8]
                    col = (ch % 8) * TILE + int(chunk_par[ch]) * D_F
                    return wt[:, col: col + D_F]

                for g, ts in enumerate(grp_tiles):
                    t0, t1 = ts[0], ts[-1] + 1
                    gw = (t1 - t0) * TILE
                    row0 = t0 * TILE
                    rows = min(gw, OWN - row0)
                    c0 = int(chunk_base[t0])
                    nch = int(chunk_base[t1] - c0)

                    # ---- one-hot scatter matrices for every chunk ----------
                    S = sp.tile([128, max_ch * TILE], f16, tag="S")
                    nc.vector.tensor_tensor(
                        S[:, : nch * TILE].rearrange("p (c e) -> p c e",
                                                     e=TILE),
                        iota_sb[:].unsqueeze(1).broadcast_to([TILE, nch, TILE]),
                        dstl_sb[:, c0: c0 + nch]
                        .unsqueeze(2)
                        .broadcast_to([TILE, nch, TILE]),
                        mybir.AluOpType.is_equal,
                    )

                    # ---- seg-sum into PSUM, one region per dst tile --------
                    ps = p_ps.tile([D_F, gw], f32, space="PSUM", tag="ps")
                    for ti, t in enumerate(ts):
                        sl = slice(ti * TILE, (ti + 1) * TILE)
                        nmm = int(chunk_base[t + 1] - chunk_base[t])
                        nc.tensor.matmul(
                            out=ps[:, sl],
                            lhsT=gcur[:, t * D_F:(t + 1) * D_F],
                            rhs=ident_sb[:],
                            start=True,
                            stop=(nmm == 0),
                        )
                        for j in range(nmm):
                            ch = int(chunk_base[t]) + j
                            scol = (ch - c0) * TILE
                            nc.tensor.matmul(
                                out=ps[:, sl],
                                lhsT=msg_lhs(ch),
                                rhs=S[:, scol: scol + TILE],
                                start=False,
                                stop=(j == nmm - 1),
                            )

                    # ---- augmented dense layer: Q = Waug^T @ [P; sigma] ----
                    paug = pp.tile([D_F + 1, gw], f16, tag="paug")
                    nc.scalar.copy(paug[0:D_F, :gw], ps[:, :gw])
                    nc.vector.tensor_copy(
                        paug[D_F: D_F + 1, 0:gw],
                        sigma_sb[:, row0: row0 + gw])
                    qs = q_ps.tile([D_F, gw], f32, space="PSUM", tag="qs")
                    nc.tensor.matmul(
                        out=qs[0:do, :gw],
                        lhsT=waug_sb[layer][:],
                        rhs=paug[:, :gw],
                        start=True,
                        stop=True,
                    )

                    if layer < 2:
                        # g' = isd^2 * relu(Q), transposed back to node-major
                        qr = qp.tile([D_F, gw], f32, tag="qr")
                        nc.scalar.activation(
                            qr[:, :gw],
                            qs[0:D_F, :gw],
                            mybir.ActivationFunctionType.Relu,
                        )
                        for ti, t in enumerate(ts):
                            qt = t_ps.tile([TILE, D_F], f32, space="PSUM",
                                           tag="qt")
                            nc.tensor.transpose(
                                out=qt[:],
                                in_=qr[:, ti * TILE:(ti + 1) * TILE],
                                identity=ident32_sb[0:D_F, 0:D_F],
                            )
                            nc.vector.tensor_scalar_mul(
                                gnxt[:, t * D_F:(t + 1) * D_F], qt[:],
                                isd2_sb[:, t: t + 1])
                            r = min(TILE, OWN - t * TILE)
                            nc.sync.dma_start(
                                out=gown_d[layer][t * TILE: t * TILE + r, :],
                                in_=gnxt[0:r, t * D_F:(t + 1) * D_F],
                            )
                    else:
                        nc.vector.tensor_copy(
                            out_sb[:, row0: row0 + rows], qs[0:1, 0:rows])

                if layer < 2:
                    nc.sync.dma_start(out=gown_d[layer][OWN:OWNP, :],
                                      in_=zero_sb[:])
                    nc.gpsimd.collective_compute(
                        "AllGather",
                        mybir.AluOpType.bypass,
                        replica_groups=rg,
                        ins=[gown_d[layer][:]],
                        outs=[gfull_d[layer][:]],
                    )

            # out = isd * Q2  (host reshapes [1, OWN] -> [OWN, 1])
            nc.vector.tensor_tensor(
                out_sb[:], out_sb[:], isdrow_sb[:], mybir.AluOpType.mult
            )
            nc.sync.dma_start(out=out_d[:], in_=out_sb[:])

    nc.compile()
    return nc


# ----------------------------------------------------------------------------
# entry point
# ----------------------------------------------------------------------------


def kernel(x, edge_index, W0, b0, W1, b1, W2, b2):
    from concourse.bass_utils import run_bass_kernel_spmd

    meta, in_maps = _prepare(x, edge_index, W0, b0, W1, b1, W2, b2)
    nc = _build(meta)
    res = run_bass_kernel_spmd(nc, in_maps, list(range(NC_CORES)))
    out = np.concatenate(
        [res.results[c]["out"].reshape(-1, 1) for c in range(NC_CORES)], axis=0
    )
    return out.astype(np.float32)
